# revision 5
# baseline (speedup 1.0000x reference)
"""NF4-quantized LoRA linear layer on 8 Trainium2 NeuronCores.

Computation (reference):
    w = NF4_TABLE[w_codes] * w_scales[block-expanded]        # [O, I]
    out = x @ w.T + (alpha/rank) * (x @ lora_a.T) @ lora_b.T # [B, S, O]

Strategy:
  - Tensor-parallel split of the output dim across 8 cores (O_SH = 512 each).
    Every core sees all of x; no collectives; host concatenates outputs.
  - The LoRA path is folded into the weights once per core:
    W_eff = dequant(codes) * scales + (alpha/rank) * lora_a.T @ lora_b.T,
    so the steady-state loop is a single dense bf16 matmul.
  - NF4 dequant uses an 8-basis approximation (linear + 4 steps + 3 ramps,
    f16-chain-tuned, max table err 4.9e-3): ACT computes Sign/Relu basis
    tiles, DVE merges them with fused scalar_tensor_tensor ops. This keeps
    both engines ~50% loaded instead of serializing everything on DVE.
  - The contraction dim runs in 4 phases (3/8/11/10 i-tiles). The running
    partial stays in SBUF as bf16 (no DRAM round trip); phase psums merge
    into it with one DVE add per m-tile, and the last phase adds + emits
    f32 straight to the output DMA.
"""

import numpy as np
import ml_dtypes

import concourse.mybir as mybir
import concourse.tile as tile
from concourse import bacc
from concourse.bass_utils import run_bass_kernel_spmd

B, S, I, O, R, BLK = 4, 2048, 4096, 4096, 16, 64
M = B * S                      # 8192 token rows
N_CORES = 8
O_SH = O // N_CORES            # 512 output cols per core
IT = I // 128                  # 32 contraction tiles
MT = M // 128                  # 64 row tiles
LORA_SCALE = 2.0               # alpha / rank

# NF4 decode: t(c) ~= G0*c + BP' + sum d_v*[c>=v] + sum g_u*relu(c-u)
# (8 basis incl. linear; constants tuned on the exact f16 chain,
#  max abs table err 4.9e-3 -> ~7e-3 worst-case rel err on the output,
#  comfortably under the 2e-2 gate)
DEC_B = -0.9998779296875
DEC_G0 = 0.3038071990013129
DEC_STEPS = [(2, 0.06993426640908616), (3, 0.03336246990628824),
             (14, 0.04890079195464007), (15, 0.1629870898660809)]
DEC_RAMPS = [(1, -0.20262171586434732), (6, -0.017353153673390995),
             (11, 0.03004063897796238)]

PHASES = [(0, 3), (3, 11), (11, 22), (22, 32)]
# dequant macro-tile i-ranges; narrow at the front for low first-weight
# latency, wide later for DVE efficiency
MACROS = [(0, 1), (1, 2), (2, 3), (3, 5), (5, 7), (7, 9), (9, 11),
          (11, 15), (15, 19), (19, 22), (22, 27), (27, 32)]
# macros emitted after each phase's loop (the rest are emitted up front);
# deadlines: a phase's tiles must decode before the NEXT phase starts
MACROS_AFTER_PHASE = {0: [(11, 15), (15, 19), (19, 22)],
                      1: [(22, 27), (27, 32)]}
N_IMMEDIATE = 7                # macros emitted before phase 0

F16 = mybir.dt.float16
BF16 = mybir.dt.bfloat16
F32 = mybir.dt.float32
ALU = mybir.AluOpType
ACTF = mybir.ActivationFunctionType

BF16_NP = ml_dtypes.bfloat16


def _build_nc():
    nc = bacc.Bacc("TRN2", target_bir_lowering=False, debug=False,
                   num_devices=N_CORES)

    xt = nc.dram_tensor("xt", [128, MT, IT, 128], BF16, kind="ExternalInput")
    codes = nc.dram_tensor("codes", [I, O_SH], F16, kind="ExternalInput")
    scales = nc.dram_tensor("scales", [I, O_SH], F16, kind="ExternalInput")
    la = nc.dram_tensor("la", [R, I], BF16, kind="ExternalInput")
    lb = nc.dram_tensor("lb", [R, O_SH], BF16, kind="ExternalInput")
    out = nc.dram_tensor("out", [M, O_SH], F32, kind="ExternalOutput")

    codes_r = codes.ap().rearrange("(t p) o -> p t o", p=128)
    scales_r = scales.ap().rearrange("(t p) o -> p t o", p=128)

    BP = DEC_B + sum(d for _, d in DEC_STEPS) / 2.0

    with tile.TileContext(nc) as tc:
        with (
            tc.tile_pool(name="wpool", bufs=1) as wpool,
            tc.tile_pool(name="wlpool", bufs=1) as wlpool,
            tc.tile_pool(name="ctp", bufs=2) as ctp,
            tc.tile_pool(name="stp", bufs=2) as stp,
            tc.tile_pool(name="sgp", bufs=4) as sgp,
            tc.tile_pool(name="accp", bufs=2) as accp,
            tc.tile_pool(name="xpool", bufs=3) as xpool,
            tc.tile_pool(name="partp", bufs=1) as partp,
            tc.tile_pool(name="cpool", bufs=1) as cpool,
            tc.tile_pool(name="opool", bufs=4) as opool,
            tc.tile_pool(name="ps", bufs=6, space="PSUM") as pp,
            tc.tile_pool(name="ps_l", bufs=2, space="PSUM") as pp_l,
        ):
            # constants
            la_sb = cpool.tile([R, I], BF16, tag="la")
            nc.sync.dma_start(la_sb[:], la.ap())
            lb_sb = cpool.tile([R, O_SH], BF16, tag="lb")
            nc.sync.dma_start(lb_sb[:], lb.ap())
            step_biases = []
            for v, _ in DEC_STEPS:
                bt = cpool.tile([128, 1], F32, tag=f"sb{v}")
                nc.vector.memset(bt[:], -(v - 0.5))
                step_biases.append(bt)
            ramp_biases = []
            for u, g in DEC_RAMPS:
                bt = cpool.tile([128, 1], F32, tag=f"rb{u}")
                nc.vector.memset(bt[:], -abs(g) * u)
                ramp_biases.append(bt)

            w_aps = {}

            def emit_macro(it_lo, it_hi):
                nt = it_hi - it_lo
                fd = nt * O_SH
                # LoRA weight fold for this macro's i-tiles
                wl = wlpool.tile([128, fd], BF16, tag=f"wl{it_lo}")
                for j, it in enumerate(range(it_lo, it_hi)):
                    pl = pp_l.tile([128, O_SH], F32, tag="pl")
                    nc.tensor.matmul(
                        pl[:], la_sb[:, it * 128:(it + 1) * 128], lb_sb[:],
                        start=True, stop=True,
                    )
                    nc.vector.tensor_scalar_mul(
                        wl[:, j * O_SH:(j + 1) * O_SH], pl[:], 1.0
                    )
                # NF4 decode
                ct = ctp.tile([128, fd], F16, tag="ct")
                nc.sync.dma_start(
                    ct[:].rearrange("p (t o) -> p t o", t=nt),
                    codes_r[:, it_lo:it_hi, :],
                )
                st = stp.tile([128, fd], F16, tag="st")
                nc.sync.dma_start(
                    st[:].rearrange("p (t o) -> p t o", t=nt),
                    scales_r[:, it_lo:it_hi, :],
                )
                acc = accp.tile([128, fd], F16, tag="acc")
                nc.vector.tensor_scalar(
                    acc[:], ct[:], DEC_G0, BP, op0=ALU.mult, op1=ALU.add
                )
                for (v, d), bt in zip(DEC_STEPS, step_biases):
                    sg = sgp.tile([128, fd], F16, tag="sg")
                    nc.scalar.activation(
                        sg[:], ct[:], ACTF.Sign, bias=bt[:], scale=1.0
                    )
                    nc.vector.scalar_tensor_tensor(
                        acc[:], sg[:], d / 2.0, acc[:],
                        op0=ALU.mult, op1=ALU.add,
                    )
                for (u, g), bt in zip(DEC_RAMPS, ramp_biases):
                    rp = sgp.tile([128, fd], F16, tag="sg")
                    nc.scalar.activation(
                        rp[:], ct[:], ACTF.Relu, bias=bt[:], scale=abs(g)
                    )
                    nc.vector.tensor_tensor(
                        acc[:], acc[:], rp[:],
                        op=ALU.add if g > 0 else ALU.subtract,
                    )
                # w = acc * scale + W_lora
                wtmp = sgp.tile([128, fd], F16, tag="sg")
                nc.vector.tensor_tensor(wtmp[:], acc[:], st[:], op=ALU.mult)
                wt = wpool.tile([128, fd], BF16, tag=f"w{it_lo}")
                nc.vector.tensor_tensor(wt[:], wtmp[:], wl[:], op=ALU.add)
                for j, it in enumerate(range(it_lo, it_hi)):
                    w_aps[it] = wt[:, j * O_SH:(j + 1) * O_SH]

            for rng in MACROS[:N_IMMEDIATE]:
                emit_macro(*rng)

            # ---- m-loop phases; partial lives in SBUF as bf16 ----
            part_tiles = [
                partp.tile([128, O_SH], BF16, tag=f"pt{mt}", name=f"pt{mt}")
                for mt in range(MT)
            ]
            n_ph = len(PHASES)
            for ph, (lo, hi) in enumerate(PHASES):
                w = hi - lo
                for mt in range(MT):
                    xa = xpool.tile([128, w, 128], BF16, tag="xa")
                    nc.sync.dma_start(xa[:], xt.ap()[:, mt, lo:hi, :])
                    po = pp.tile([128, O_SH], F32, tag="po")
                    for k, it in enumerate(range(lo, hi)):
                        nc.tensor.matmul(
                            po[:], xa[:, k, :], w_aps[it],
                            start=(k == 0), stop=(k == w - 1),
                        )
                    pt = part_tiles[mt]
                    if ph == 0:
                        # split psum evacuation across DVE and ACT so
                        # neither throttles the short phase-0 groups
                        if mt % 3 == 0:
                            nc.vector.tensor_scalar_mul(pt[:], po[:], 1.0)
                        else:
                            nc.scalar.copy(pt[:], po[:])
                    elif ph < n_ph - 1:
                        nc.vector.tensor_tensor(
                            pt[:], po[:], pt[:], op=ALU.add
                        )
                    else:
                        ev = opool.tile([128, O_SH], F32, tag="ev")
                        nc.vector.tensor_tensor(
                            ev[:], po[:], pt[:], op=ALU.add
                        )
                        nc.sync.dma_start(
                            out.ap()[mt * 128:(mt + 1) * 128, :], ev[:]
                        )
                for rng in MACROS_AFTER_PHASE.get(ph, []):
                    emit_macro(*rng)

    nc.compile()
    return nc


_NC_CACHE = {}


def _get_nc():
    if "nc" not in _NC_CACHE:
        _NC_CACHE["nc"] = _build_nc()
    return _NC_CACHE["nc"]


def prepare_in_maps(x, w_codes, w_scales, lora_a, lora_b):
    """Host-side sharding + layout prep (no arithmetic beyond casts/folds)."""
    xm = np.ascontiguousarray(x.reshape(M, I))
    # xt[p, mt, t, mm] = x[mt*128+mm, t*128+p], bf16
    xtl = (
        xm.T.reshape(IT, 128, MT, 128)
        .transpose(1, 2, 0, 3)
        .astype(BF16_NP)
    )
    xtl = np.ascontiguousarray(xtl)

    la = np.ascontiguousarray(
        (LORA_SCALE * lora_a.astype(np.float64)).astype(BF16_NP)
    )

    in_maps = []
    for c in range(N_CORES):
        o_lo, o_hi = c * O_SH, (c + 1) * O_SH
        codes_t = np.ascontiguousarray(
            w_codes[o_lo:o_hi].T.astype(np.float16)
        )
        scales_t = np.ascontiguousarray(
            np.repeat(w_scales[o_lo:o_hi].T, BLK, axis=0).astype(np.float16)
        )
        lb_t = np.ascontiguousarray(lora_b[o_lo:o_hi].T.astype(BF16_NP))
        in_maps.append(
            {
                "xt": xtl,
                "codes": codes_t,
                "scales": scales_t,
                "la": la,
                "lb": lb_t,
            }
        )
    return in_maps


def run(in_maps, trace=False, retries=2):
    nc = _get_nc()
    last = None
    for attempt in range(retries + 1):
        try:
            return run_bass_kernel_spmd(
                nc, in_maps, core_ids=list(range(N_CORES)), trace=trace
            )
        except Exception as e:  # transient NRT/axon device errors
            last = e
            if attempt == retries:
                raise
            import time as _time

            _time.sleep(5)
    raise last


def kernel(x, w_codes, w_scales, lora_a, lora_b):
    in_maps = prepare_in_maps(x, w_codes, w_scales, lora_a, lora_b)
    res = run(in_maps, trace=False)
    out = np.concatenate(
        [res.results[c]["out"] for c in range(N_CORES)], axis=1
    )
    return out.reshape(B, S, O).astype(np.float32)


# revision 9
# speedup vs baseline: 1.1108x; 1.1108x over previous
"""NF4-quantized LoRA linear layer on 8 Trainium2 NeuronCores.

Computation (reference):
    w = NF4_TABLE[w_codes] * w_scales[block-expanded]        # [O, I]
    out = x @ w.T + (alpha/rank) * (x @ lora_a.T) @ lora_b.T # [B, S, O]

Strategy:
  - Tensor-parallel split of the output dim across 8 cores (O_SH = 512 each).
    Every core sees all of x; no collectives; host concatenates outputs.
  - The LoRA path is folded into the weights once per core:
    W_eff = dequant(codes) * scales + (alpha/rank) * lora_a.T @ lora_b.T,
    so the steady-state loop is a single dense bf16 matmul.
  - NF4 dequant uses an 8-basis approximation (linear + 4 steps + 3 ramps,
    f16-chain-tuned, max table err 4.9e-3): ACT computes Sign/Relu basis
    tiles, DVE merges them with fused scalar_tensor_tensor ops. This keeps
    both engines ~50% loaded instead of serializing everything on DVE.
  - The contraction dim runs in 4 phases (3/8/11/10 i-tiles). The running
    partial stays in SBUF as bf16 (no DRAM round trip); phase psums merge
    into it with one DVE add per m-tile, and the last phase adds + emits
    f32 straight to the output DMA.
"""

import numpy as np
import ml_dtypes

import concourse.mybir as mybir
import concourse.tile as tile
from concourse import bacc
from concourse.bass_utils import run_bass_kernel_spmd

B, S, I, O, R, BLK = 4, 2048, 4096, 4096, 16, 64
M = B * S                      # 8192 token rows
N_CORES = 8
O_SH = O // N_CORES            # 512 output cols per core
IT = I // 128                  # 32 contraction tiles
MT = M // 128                  # 64 row tiles
LORA_SCALE = 2.0               # alpha / rank

# NF4 decode: t(c) ~= G0*c + BP' + sum d_v*[c>=v] + sum g_u*relu(c-u)
# (8 basis incl. linear; constants tuned on the exact f16 chain,
#  max abs table err 4.9e-3 -> ~7e-3 worst-case rel err on the output,
#  comfortably under the 2e-2 gate)
DEC_B = -0.9998779296875
DEC_G0 = 0.3038071990013129
DEC_STEPS = [(2, 0.06993426640908616), (3, 0.03336246990628824),
             (14, 0.04890079195464007), (15, 0.1629870898660809)]
DEC_RAMPS = [(1, -0.20262171586434732), (6, -0.017353153673390995),
             (11, 0.03004063897796238)]

PHASES = [(0, 6), (6, 19), (19, 32)]
# dequant macro-tile i-ranges; narrow at the front for low first-weight
# latency, wide later for DVE efficiency
MACROS = [(0, 1), (1, 2), (2, 4), (4, 6),
          (6, 10), (10, 14), (14, 17), (17, 19),
          (19, 24), (24, 28), (28, 32)]
# macros emitted after each phase's loop (the rest are emitted up front);
# deadlines: a phase's tiles must decode before that phase STARTS, so
# tiles 6-18 decode during phase 0 and 19-31 during phase 1
MACROS_AFTER_PHASE = {0: [(6, 10), (10, 14), (14, 17), (17, 19)],
                      1: [(19, 24), (24, 28), (28, 32)]}
N_IMMEDIATE = 4                # macros emitted before phase 0

F16 = mybir.dt.float16
BF16 = mybir.dt.bfloat16
F32 = mybir.dt.float32
ALU = mybir.AluOpType
ACTF = mybir.ActivationFunctionType

BF16_NP = ml_dtypes.bfloat16


def _build_nc():
    nc = bacc.Bacc("TRN2", target_bir_lowering=False, debug=False,
                   num_devices=N_CORES)

    xt = nc.dram_tensor("xt", [128, MT, IT, 128], BF16, kind="ExternalInput")
    codes = nc.dram_tensor("codes", [I, O_SH], F16, kind="ExternalInput")
    scales = nc.dram_tensor("scales", [I, O_SH], F16, kind="ExternalInput")
    la = nc.dram_tensor("la", [R, I], BF16, kind="ExternalInput")
    lb = nc.dram_tensor("lb", [R, O_SH], BF16, kind="ExternalInput")
    out = nc.dram_tensor("out", [M, O_SH], F32, kind="ExternalOutput")

    codes_r = codes.ap().rearrange("(t p) o -> p t o", p=128)
    scales_r = scales.ap().rearrange("(t p) o -> p t o", p=128)

    BP = DEC_B + sum(d for _, d in DEC_STEPS) / 2.0

    with tile.TileContext(nc) as tc:
        with (
            tc.tile_pool(name="wpool", bufs=1) as wpool,
            tc.tile_pool(name="wlpool", bufs=1) as wlpool,
            tc.tile_pool(name="ctp", bufs=2) as ctp,
            tc.tile_pool(name="stp", bufs=2) as stp,
            tc.tile_pool(name="sgp", bufs=4) as sgp,
            tc.tile_pool(name="accp", bufs=2) as accp,
            tc.tile_pool(name="xpool", bufs=4) as xpool,
            tc.tile_pool(name="partp", bufs=1) as partp,
            tc.tile_pool(name="cpool", bufs=1) as cpool,
            tc.tile_pool(name="opool", bufs=3) as opool,
            tc.tile_pool(name="ps", bufs=6, space="PSUM") as pp,
            tc.tile_pool(name="ps_l", bufs=2, space="PSUM") as pp_l,
        ):
            # constants
            la_sb = cpool.tile([R, I], BF16, tag="la")
            nc.sync.dma_start(la_sb[:], la.ap())
            lb_sb = cpool.tile([R, O_SH], BF16, tag="lb")
            nc.sync.dma_start(lb_sb[:], lb.ap())
            step_biases = []
            for v, _ in DEC_STEPS:
                bt = cpool.tile([128, 1], F32, tag=f"sb{v}")
                nc.vector.memset(bt[:], -(v - 0.5))
                step_biases.append(bt)
            ramp_biases = []
            for u, g in DEC_RAMPS:
                bt = cpool.tile([128, 1], F32, tag=f"rb{u}")
                nc.vector.memset(bt[:], -abs(g) * u)
                ramp_biases.append(bt)

            w_aps = {}

            def emit_macro(it_lo, it_hi):
                nt = it_hi - it_lo
                fd = nt * O_SH
                # LoRA weight fold for this macro's i-tiles
                wl = wlpool.tile([128, fd], BF16, tag=f"wl{it_lo}")
                for j, it in enumerate(range(it_lo, it_hi)):
                    pl = pp_l.tile([128, O_SH], F32, tag="pl")
                    nc.tensor.matmul(
                        pl[:], la_sb[:, it * 128:(it + 1) * 128], lb_sb[:],
                        start=True, stop=True,
                    )
                    nc.vector.tensor_scalar_mul(
                        wl[:, j * O_SH:(j + 1) * O_SH], pl[:], 1.0
                    )
                # NF4 decode
                ct = ctp.tile([128, fd], F16, tag="ct")
                nc.sync.dma_start(
                    ct[:].rearrange("p (t o) -> p t o", t=nt),
                    codes_r[:, it_lo:it_hi, :],
                )
                st = stp.tile([128, fd], F16, tag="st")
                nc.sync.dma_start(
                    st[:].rearrange("p (t o) -> p t o", t=nt),
                    scales_r[:, it_lo:it_hi, :],
                )
                acc = accp.tile([128, fd], F16, tag="acc")
                nc.vector.tensor_scalar(
                    acc[:], ct[:], DEC_G0, BP, op0=ALU.mult, op1=ALU.add
                )
                for (v, d), bt in zip(DEC_STEPS, step_biases):
                    sg = sgp.tile([128, fd], F16, tag="sg")
                    nc.scalar.activation(
                        sg[:], ct[:], ACTF.Sign, bias=bt[:], scale=1.0
                    )
                    nc.vector.scalar_tensor_tensor(
                        acc[:], sg[:], d / 2.0, acc[:],
                        op0=ALU.mult, op1=ALU.add,
                    )
                for (u, g), bt in zip(DEC_RAMPS, ramp_biases):
                    rp = sgp.tile([128, fd], F16, tag="sg")
                    nc.scalar.activation(
                        rp[:], ct[:], ACTF.Relu, bias=bt[:], scale=abs(g)
                    )
                    nc.vector.tensor_tensor(
                        acc[:], acc[:], rp[:],
                        op=ALU.add if g > 0 else ALU.subtract,
                    )
                # w = acc * scale + W_lora
                wtmp = sgp.tile([128, fd], F16, tag="sg")
                nc.vector.tensor_tensor(wtmp[:], acc[:], st[:], op=ALU.mult)
                wt = wpool.tile([128, fd], BF16, tag=f"w{it_lo}")
                nc.vector.tensor_tensor(wt[:], wtmp[:], wl[:], op=ALU.add)
                for j, it in enumerate(range(it_lo, it_hi)):
                    w_aps[it] = wt[:, j * O_SH:(j + 1) * O_SH]

            for rng in MACROS[:N_IMMEDIATE]:
                emit_macro(*rng)

            # ---- m-loop phases; partial lives in SBUF as bf16 ----
            part_tiles = [
                partp.tile([128, O_SH], BF16, tag=f"pt{mt}", name=f"pt{mt}")
                for mt in range(MT)
            ]
            n_ph = len(PHASES)
            for ph, (lo, hi) in enumerate(PHASES):
                w = hi - lo
                for mt in range(MT):
                    xa = xpool.tile([128, w, 128], BF16, tag="xa")
                    nc.sync.dma_start(xa[:], xt.ap()[:, mt, lo:hi, :])
                    po = pp.tile([128, O_SH], F32, tag="po")
                    for k, it in enumerate(range(lo, hi)):
                        nc.tensor.matmul(
                            po[:], xa[:, k, :], w_aps[it],
                            start=(k == 0), stop=(k == w - 1),
                        )
                    pt = part_tiles[mt]
                    if ph == 0:
                        # split psum evacuation across DVE and ACT so
                        # neither throttles the phase-0 groups
                        if mt % 2 == 0:
                            nc.vector.tensor_scalar_mul(pt[:], po[:], 1.0)
                        else:
                            nc.scalar.copy(pt[:], po[:])
                    elif ph < n_ph - 1:
                        nc.vector.tensor_tensor(
                            pt[:], po[:], pt[:], op=ALU.add
                        )
                    else:
                        ev = opool.tile([128, O_SH], F32, tag="ev")
                        nc.vector.tensor_tensor(
                            ev[:], po[:], pt[:], op=ALU.add
                        )
                        nc.sync.dma_start(
                            out.ap()[mt * 128:(mt + 1) * 128, :], ev[:]
                        )
                for rng in MACROS_AFTER_PHASE.get(ph, []):
                    emit_macro(*rng)

    nc.compile()
    return nc


_NC_CACHE = {}


def _get_nc():
    if "nc" not in _NC_CACHE:
        _NC_CACHE["nc"] = _build_nc()
    return _NC_CACHE["nc"]


def prepare_in_maps(x, w_codes, w_scales, lora_a, lora_b):
    """Host-side sharding + layout prep (no arithmetic beyond casts/folds)."""
    xm = np.ascontiguousarray(x.reshape(M, I))
    # xt[p, mt, t, mm] = x[mt*128+mm, t*128+p], bf16
    xtl = (
        xm.T.reshape(IT, 128, MT, 128)
        .transpose(1, 2, 0, 3)
        .astype(BF16_NP)
    )
    xtl = np.ascontiguousarray(xtl)

    la = np.ascontiguousarray(
        (LORA_SCALE * lora_a.astype(np.float64)).astype(BF16_NP)
    )

    in_maps = []
    for c in range(N_CORES):
        o_lo, o_hi = c * O_SH, (c + 1) * O_SH
        codes_t = np.ascontiguousarray(
            w_codes[o_lo:o_hi].T.astype(np.float16)
        )
        scales_t = np.ascontiguousarray(
            np.repeat(w_scales[o_lo:o_hi].T, BLK, axis=0).astype(np.float16)
        )
        lb_t = np.ascontiguousarray(lora_b[o_lo:o_hi].T.astype(BF16_NP))
        in_maps.append(
            {
                "xt": xtl,
                "codes": codes_t,
                "scales": scales_t,
                "la": la,
                "lb": lb_t,
            }
        )
    return in_maps


def run(in_maps, trace=False, retries=2):
    nc = _get_nc()
    last = None
    for attempt in range(retries + 1):
        try:
            return run_bass_kernel_spmd(
                nc, in_maps, core_ids=list(range(N_CORES)), trace=trace
            )
        except Exception as e:  # transient NRT/axon device errors
            last = e
            if attempt == retries:
                raise
            import time as _time

            _time.sleep(5)
    raise last


def kernel(x, w_codes, w_scales, lora_a, lora_b):
    in_maps = prepare_in_maps(x, w_codes, w_scales, lora_a, lora_b)
    res = run(in_maps, trace=False)
    out = np.concatenate(
        [res.results[c]["out"] for c in range(N_CORES)], axis=1
    )
    return out.reshape(B, S, O).astype(np.float32)


# revision 13
# speedup vs baseline: 1.1260x; 1.0138x over previous
"""NF4-quantized LoRA linear layer on 8 Trainium2 NeuronCores.

Computation (reference):
    w = NF4_TABLE[w_codes] * w_scales[block-expanded]        # [O, I]
    out = x @ w.T + (alpha/rank) * (x @ lora_a.T) @ lora_b.T # [B, S, O]

Strategy:
  - Tensor-parallel split of the output dim across 8 cores (O_SH = 512 each).
    Every core sees all of x; no collectives; host concatenates outputs.
  - The LoRA path is folded into the weights once per core:
    W_eff = dequant(codes) * scales + (alpha/rank) * lora_a.T @ lora_b.T,
    so the steady-state loop is a single dense bf16 matmul.
  - NF4 dequant uses an 8-basis approximation (linear + 4 steps + 3 ramps,
    f16-chain-tuned, max table err 4.9e-3): ACT computes Sign/Relu basis
    tiles, DVE merges them with fused scalar_tensor_tensor ops. This keeps
    both engines ~50% loaded instead of serializing everything on DVE.
  - The contraction dim runs in 4 phases (3/8/11/10 i-tiles). The running
    partial stays in SBUF as bf16 (no DRAM round trip); phase psums merge
    into it with one DVE add per m-tile, and the last phase adds + emits
    f32 straight to the output DMA.
"""

import numpy as np
import ml_dtypes

import concourse.mybir as mybir
import concourse.tile as tile
from concourse import bacc
from concourse.bass_utils import run_bass_kernel_spmd

B, S, I, O, R, BLK = 4, 2048, 4096, 4096, 16, 64
M = B * S                      # 8192 token rows
N_CORES = 8
O_SH = O // N_CORES            # 512 output cols per core
IT = I // 128                  # 32 contraction tiles
MT = M // 128                  # 64 row tiles
LORA_SCALE = 2.0               # alpha / rank

# NF4 decode: t(c) ~= G0*c + BP' + sum d_v*[c>=v] + sum g_u*relu(c-u)
# (8 basis incl. linear; constants tuned on the exact f16 chain,
#  max abs table err 4.9e-3 -> ~7e-3 worst-case rel err on the output,
#  comfortably under the 2e-2 gate)
DEC_B = -0.9998779296875
DEC_G0 = 0.3038071990013129
DEC_STEPS = [(2, 0.06993426640908616), (3, 0.03336246990628824),
             (14, 0.04890079195464007), (15, 0.1629870898660809)]
DEC_RAMPS = [(1, -0.20262171586434732), (6, -0.017353153673390995),
             (11, 0.03004063897796238)]

PHASES = [(0, 6), (6, 19), (19, 32)]
# dequant macro-tile i-ranges; narrow at the front for low first-weight
# latency, wide later for DVE efficiency
MACROS = [(0, 1), (1, 2), (2, 4), (4, 6),
          (6, 10), (10, 14), (14, 17), (17, 19),
          (19, 24), (24, 28), (28, 32)]
# macros emitted after each phase's loop (the rest are emitted up front);
# deadlines: a phase's tiles must decode before that phase STARTS, so
# tiles 6-18 decode during phase 0 and 19-31 during phase 1
MACROS_AFTER_PHASE = {0: [(6, 10), (10, 14), (14, 17), (17, 19)],
                      1: [(19, 24), (24, 28), (28, 32)]}
N_IMMEDIATE = 4                # macros emitted before phase 0

F16 = mybir.dt.float16
BF16 = mybir.dt.bfloat16
F32 = mybir.dt.float32
ALU = mybir.AluOpType
ACTF = mybir.ActivationFunctionType

BF16_NP = ml_dtypes.bfloat16


def _build_nc():
    nc = bacc.Bacc("TRN2", target_bir_lowering=False, debug=False,
                   num_devices=N_CORES)

    xt = nc.dram_tensor("xt", [128, MT, IT, 128], BF16, kind="ExternalInput")
    codes = nc.dram_tensor("codes", [I, O_SH], F16, kind="ExternalInput")
    scales = nc.dram_tensor("scales", [I, O_SH], F16, kind="ExternalInput")
    la = nc.dram_tensor("la", [R, I], BF16, kind="ExternalInput")
    lb = nc.dram_tensor("lb", [R, O_SH], BF16, kind="ExternalInput")
    out = nc.dram_tensor("out", [M, O_SH], F32, kind="ExternalOutput")

    codes_r = codes.ap().rearrange("(t p) o -> p t o", p=128)
    scales_r = scales.ap().rearrange("(t p) o -> p t o", p=128)

    BP = DEC_B + sum(d for _, d in DEC_STEPS) / 2.0

    with tile.TileContext(nc) as tc:
        with (
            tc.tile_pool(name="wpool", bufs=1) as wpool,
            tc.tile_pool(name="wlpool", bufs=1) as wlpool,
            tc.tile_pool(name="ctp", bufs=2) as ctp,
            tc.tile_pool(name="stp", bufs=2) as stp,
            tc.tile_pool(name="sgp", bufs=4) as sgp,
            tc.tile_pool(name="accp", bufs=2) as accp,
            tc.tile_pool(name="xpool", bufs=4) as xpool,
            tc.tile_pool(name="partp", bufs=1) as partp,
            tc.tile_pool(name="cpool", bufs=1) as cpool,
            tc.tile_pool(name="opool", bufs=3) as opool,
            tc.tile_pool(name="ps", bufs=6, space="PSUM") as pp,
            tc.tile_pool(name="ps_l", bufs=2, space="PSUM") as pp_l,
        ):
            # constants
            la_sb = cpool.tile([R, I], BF16, tag="la")
            nc.sync.dma_start(la_sb[:], la.ap())
            lb_sb = cpool.tile([R, O_SH], BF16, tag="lb")
            nc.sync.dma_start(lb_sb[:], lb.ap())
            step_biases = []
            for v, _ in DEC_STEPS:
                bt = cpool.tile([128, 1], F32, tag=f"sb{v}")
                nc.vector.memset(bt[:], -(v - 0.5))
                step_biases.append(bt)
            ramp_biases = []
            for u, g in DEC_RAMPS:
                bt = cpool.tile([128, 1], F32, tag=f"rb{u}")
                nc.vector.memset(bt[:], -abs(g) * u)
                ramp_biases.append(bt)

            w_aps = {}

            # ---- LoRA fold, all upfront: warms the PE before phase 0 and
            # keeps fold matmuls out of the steady-state PE stream ----
            wl_tiles = {}
            for mi, (it_lo, it_hi) in enumerate(MACROS):
                wl = wlpool.tile(
                    [128, (it_hi - it_lo) * O_SH], BF16,
                    tag=f"wl{it_lo}", name=f"wl{it_lo}",
                )
                wl_tiles[it_lo] = wl
                for j, it in enumerate(range(it_lo, it_hi)):
                    pl = pp_l.tile([128, O_SH], F32, tag="pl", name="pl")
                    nc.tensor.matmul(
                        pl[:], la_sb[:, it * 128:(it + 1) * 128], lb_sb[:],
                        start=True, stop=True,
                    )
                    dst = wl[:, j * O_SH:(j + 1) * O_SH]
                    if it % 2 == 0:
                        nc.vector.tensor_scalar_mul(dst, pl[:], 1.0)
                    else:
                        nc.scalar.copy(dst, pl[:])

            def emit_macro(it_lo, it_hi):
                nt = it_hi - it_lo
                fd = nt * O_SH
                wl = wl_tiles[it_lo]
                # NF4 decode
                ct = ctp.tile([128, fd], F16, tag="ct")
                nc.sync.dma_start(
                    ct[:].rearrange("p (t o) -> p t o", t=nt),
                    codes_r[:, it_lo:it_hi, :],
                )
                st = stp.tile([128, fd], F16, tag="st")
                nc.sync.dma_start(
                    st[:].rearrange("p (t o) -> p t o", t=nt),
                    scales_r[:, it_lo:it_hi, :],
                )
                acc = accp.tile([128, fd], F16, tag="acc")
                # sign-form steps carry a +d/2 offset folded into the init
                init_b = BP if nt <= 2 else DEC_B
                nc.vector.tensor_scalar(
                    acc[:], ct[:], DEC_G0, init_b, op0=ALU.mult, op1=ALU.add
                )
                if nt <= 2:
                    # narrow early macros: ACT Sign + DVE stt keeps the
                    # serial chain short (stt is cheap at small fd)
                    for (v, d), bt in zip(DEC_STEPS, step_biases):
                        sg = sgp.tile([128, fd], F16, tag="sg")
                        nc.scalar.activation(
                            sg[:], ct[:], ACTF.Sign, bias=bt[:], scale=1.0
                        )
                        nc.vector.scalar_tensor_tensor(
                            acc[:], sg[:], d / 2.0, acc[:],
                            op0=ALU.mult, op1=ALU.add,
                        )
                else:
                    # wide macros: plain is_ge+mult then add — stt runs at
                    # ~0.75 elem/cycle on wide tiles, 2x slower than this
                    for v, d in DEC_STEPS:
                        sg = sgp.tile([128, fd], F16, tag="sg")
                        nc.vector.tensor_scalar(
                            sg[:], ct[:], v - 0.5, d,
                            op0=ALU.is_ge, op1=ALU.mult,
                        )
                        nc.vector.tensor_tensor(
                            acc[:], acc[:], sg[:], op=ALU.add
                        )
                for (u, g), bt in zip(DEC_RAMPS, ramp_biases):
                    rp = sgp.tile([128, fd], F16, tag="sg")
                    nc.scalar.activation(
                        rp[:], ct[:], ACTF.Relu, bias=bt[:], scale=abs(g)
                    )
                    nc.vector.tensor_tensor(
                        acc[:], acc[:], rp[:],
                        op=ALU.add if g > 0 else ALU.subtract,
                    )
                # w = acc * scale + W_lora
                wtmp = sgp.tile([128, fd], F16, tag="sg")
                nc.vector.tensor_tensor(wtmp[:], acc[:], st[:], op=ALU.mult)
                wt = wpool.tile([128, fd], BF16, tag=f"w{it_lo}")
                nc.vector.tensor_tensor(wt[:], wtmp[:], wl[:], op=ALU.add)
                for j, it in enumerate(range(it_lo, it_hi)):
                    w_aps[it] = wt[:, j * O_SH:(j + 1) * O_SH]

            for rng in MACROS[:N_IMMEDIATE]:
                emit_macro(*rng)

            # ---- m-loop phases; partial lives in SBUF as bf16 ----
            part_tiles = [
                partp.tile([128, O_SH], BF16, tag=f"pt{mt}", name=f"pt{mt}")
                for mt in range(MT)
            ]
            n_ph = len(PHASES)
            for ph, (lo, hi) in enumerate(PHASES):
                w = hi - lo
                for mt in range(MT):
                    xa = xpool.tile([128, w, 128], BF16, tag="xa")
                    nc.sync.dma_start(xa[:], xt.ap()[:, mt, lo:hi, :])
                    po = pp.tile([128, O_SH], F32, tag="po")
                    for k, it in enumerate(range(lo, hi)):
                        nc.tensor.matmul(
                            po[:], xa[:, k, :], w_aps[it],
                            start=(k == 0), stop=(k == w - 1),
                        )
                    pt = part_tiles[mt]
                    if ph == 0:
                        # ACT handles all phase-0 evacuation (DVE carries
                        # the decode chain during this window)
                        nc.scalar.copy(pt[:], po[:])
                    elif ph < n_ph - 1:
                        nc.vector.tensor_tensor(
                            pt[:], po[:], pt[:], op=ALU.add
                        )
                    else:
                        ev = opool.tile([128, O_SH], F32, tag="ev")
                        nc.vector.tensor_tensor(
                            ev[:], po[:], pt[:], op=ALU.add
                        )
                        nc.sync.dma_start(
                            out.ap()[mt * 128:(mt + 1) * 128, :], ev[:]
                        )
                for rng in MACROS_AFTER_PHASE.get(ph, []):
                    emit_macro(*rng)

    nc.compile()
    return nc


_NC_CACHE = {}


def _get_nc():
    if "nc" not in _NC_CACHE:
        _NC_CACHE["nc"] = _build_nc()
    return _NC_CACHE["nc"]


def prepare_in_maps(x, w_codes, w_scales, lora_a, lora_b):
    """Host-side sharding + layout prep (no arithmetic beyond casts/folds)."""
    xm = np.ascontiguousarray(x.reshape(M, I))
    # xt[p, mt, t, mm] = x[mt*128+mm, t*128+p], bf16
    xtl = (
        xm.T.reshape(IT, 128, MT, 128)
        .transpose(1, 2, 0, 3)
        .astype(BF16_NP)
    )
    xtl = np.ascontiguousarray(xtl)

    la = np.ascontiguousarray(
        (LORA_SCALE * lora_a.astype(np.float64)).astype(BF16_NP)
    )

    in_maps = []
    for c in range(N_CORES):
        o_lo, o_hi = c * O_SH, (c + 1) * O_SH
        codes_t = np.ascontiguousarray(
            w_codes[o_lo:o_hi].T.astype(np.float16)
        )
        scales_t = np.ascontiguousarray(
            np.repeat(w_scales[o_lo:o_hi].T, BLK, axis=0).astype(np.float16)
        )
        lb_t = np.ascontiguousarray(lora_b[o_lo:o_hi].T.astype(BF16_NP))
        in_maps.append(
            {
                "xt": xtl,
                "codes": codes_t,
                "scales": scales_t,
                "la": la,
                "lb": lb_t,
            }
        )
    return in_maps


def run(in_maps, trace=False, retries=2):
    nc = _get_nc()
    last = None
    for attempt in range(retries + 1):
        try:
            return run_bass_kernel_spmd(
                nc, in_maps, core_ids=list(range(N_CORES)), trace=trace
            )
        except Exception as e:  # transient NRT/axon device errors
            last = e
            if attempt == retries:
                raise
            import time as _time

            _time.sleep(5)
    raise last


def kernel(x, w_codes, w_scales, lora_a, lora_b):
    in_maps = prepare_in_maps(x, w_codes, w_scales, lora_a, lora_b)
    res = run(in_maps, trace=False)
    out = np.concatenate(
        [res.results[c]["out"] for c in range(N_CORES)], axis=1
    )
    return out.reshape(B, S, O).astype(np.float32)


# revision 15
# speedup vs baseline: 1.1352x; 1.0081x over previous
"""NF4-quantized LoRA linear layer on 8 Trainium2 NeuronCores.

Computation (reference):
    w = NF4_TABLE[w_codes] * w_scales[block-expanded]        # [O, I]
    out = x @ w.T + (alpha/rank) * (x @ lora_a.T) @ lora_b.T # [B, S, O]

Strategy:
  - Tensor-parallel split of the output dim across 8 cores (O_SH = 512 each).
    Every core sees all of x; no collectives; host concatenates outputs.
  - The LoRA path is folded into the weights once per core:
    W_eff = dequant(codes) * scales + (alpha/rank) * lora_a.T @ lora_b.T,
    so the steady-state loop is a single dense bf16 matmul.
  - NF4 dequant uses an 8-basis approximation (linear + 4 steps + 3 ramps,
    f16-chain-tuned, max table err 4.9e-3): ACT computes Sign/Relu basis
    tiles, DVE merges them with fused scalar_tensor_tensor ops. This keeps
    both engines ~50% loaded instead of serializing everything on DVE.
  - The contraction dim runs in 4 phases (3/8/11/10 i-tiles). The running
    partial stays in SBUF as bf16 (no DRAM round trip); phase psums merge
    into it with one DVE add per m-tile, and the last phase adds + emits
    f32 straight to the output DMA.
"""

import numpy as np
import ml_dtypes

import concourse.mybir as mybir
import concourse.tile as tile
from concourse import bacc
from concourse.bass_utils import run_bass_kernel_spmd

B, S, I, O, R, BLK = 4, 2048, 4096, 4096, 16, 64
M = B * S                      # 8192 token rows
N_CORES = 8
O_SH = O // N_CORES            # 512 output cols per core
IT = I // 128                  # 32 contraction tiles
MT = M // 128                  # 64 row tiles
LORA_SCALE = 2.0               # alpha / rank

# NF4 decode: t(c) ~= G0*c + BP' + sum d_v*[c>=v] + sum g_u*relu(c-u)
# (8 basis incl. linear; constants tuned on the exact f16 chain,
#  max abs table err 4.9e-3 -> ~7e-3 worst-case rel err on the output,
#  comfortably under the 2e-2 gate)
DEC_B = -0.9998779296875
DEC_G0 = 0.3038071990013129
DEC_STEPS = [(2, 0.06993426640908616), (3, 0.03336246990628824),
             (14, 0.04890079195464007), (15, 0.1629870898660809)]
DEC_RAMPS = [(1, -0.20262171586434732), (6, -0.017353153673390995),
             (11, 0.03004063897796238)]

PHASES = [(0, 6), (6, 19), (19, 32)]
# dequant macro-tile i-ranges; narrow at the front for low first-weight
# latency, wide later for DVE efficiency
MACROS = [(0, 1), (1, 2), (2, 4), (4, 6),
          (6, 10), (10, 14), (14, 17), (17, 19),
          (19, 24), (24, 28), (28, 32)]
# macros emitted after each phase's loop (the rest are emitted up front);
# deadlines: a phase's tiles must decode before that phase STARTS, so
# tiles 6-18 decode during phase 0 and 19-31 during phase 1
MACROS_AFTER_PHASE = {0: [(6, 10), (10, 14), (14, 17), (17, 19)],
                      1: [(19, 24), (24, 28), (28, 32)]}
N_IMMEDIATE = 4                # macros emitted before phase 0

F16 = mybir.dt.float16
BF16 = mybir.dt.bfloat16
F32 = mybir.dt.float32
ALU = mybir.AluOpType
ACTF = mybir.ActivationFunctionType

BF16_NP = ml_dtypes.bfloat16


def _build_nc():
    nc = bacc.Bacc("TRN2", target_bir_lowering=False, debug=False,
                   num_devices=N_CORES)

    xt = nc.dram_tensor("xt", [128, MT, IT, 128], BF16, kind="ExternalInput")
    codes = nc.dram_tensor("codes", [I, O_SH], F16, kind="ExternalInput")
    scales = nc.dram_tensor("scales", [I, O_SH], F16, kind="ExternalInput")
    la = nc.dram_tensor("la", [R, I], BF16, kind="ExternalInput")
    lb = nc.dram_tensor("lb", [R, O_SH], BF16, kind="ExternalInput")
    out = nc.dram_tensor("out", [M, O_SH], F32, kind="ExternalOutput")

    codes_r = codes.ap().rearrange("(t p) o -> p t o", p=128)
    scales_r = scales.ap().rearrange("(t p) o -> p t o", p=128)

    BP = DEC_B + sum(d for _, d in DEC_STEPS) / 2.0

    with tile.TileContext(nc) as tc:
        with (
            tc.tile_pool(name="wpool", bufs=1) as wpool,
            tc.tile_pool(name="wlpool", bufs=1) as wlpool,
            tc.tile_pool(name="ctp", bufs=2) as ctp,
            tc.tile_pool(name="stp", bufs=2) as stp,
            tc.tile_pool(name="sgp", bufs=4) as sgp,
            tc.tile_pool(name="accp", bufs=2) as accp,
            tc.tile_pool(name="xpool", bufs=4) as xpool,
            tc.tile_pool(name="partp", bufs=1) as partp,
            tc.tile_pool(name="cpool", bufs=1) as cpool,
            tc.tile_pool(name="opool", bufs=3) as opool,
            tc.tile_pool(name="ps", bufs=6, space="PSUM") as pp,
            tc.tile_pool(name="ps_l", bufs=2, space="PSUM") as pp_l,
        ):
            # constants
            la_sb = cpool.tile([R, I], BF16, tag="la")
            nc.sync.dma_start(la_sb[:], la.ap())
            lb_sb = cpool.tile([R, O_SH], BF16, tag="lb")
            nc.sync.dma_start(lb_sb[:], lb.ap())
            ramp_biases = []
            for u, g in DEC_RAMPS:
                bt = cpool.tile([128, 1], F32, tag=f"rb{u}")
                nc.vector.memset(bt[:], -abs(g) * u)
                ramp_biases.append(bt)

            w_aps = {}

            # ---- LoRA fold, all upfront: warms the PE before phase 0 and
            # keeps fold matmuls out of the steady-state PE stream ----
            wl_tiles = {}
            for mi, (it_lo, it_hi) in enumerate(MACROS):
                wl = wlpool.tile(
                    [128, (it_hi - it_lo) * O_SH], BF16,
                    tag=f"wl{it_lo}", name=f"wl{it_lo}",
                )
                wl_tiles[it_lo] = wl
                for j, it in enumerate(range(it_lo, it_hi)):
                    pl = pp_l.tile([128, O_SH], F32, tag="pl", name="pl")
                    nc.tensor.matmul(
                        pl[:], la_sb[:, it * 128:(it + 1) * 128], lb_sb[:],
                        start=True, stop=True,
                    )
                    dst = wl[:, j * O_SH:(j + 1) * O_SH]
                    if it % 2 == 0:
                        nc.vector.tensor_scalar_mul(dst, pl[:], 1.0)
                    else:
                        nc.scalar.copy(dst, pl[:])

            def emit_macro(it_lo, it_hi):
                nt = it_hi - it_lo
                fd = nt * O_SH
                wl = wl_tiles[it_lo]
                # NF4 decode
                ct = ctp.tile([128, fd], F16, tag="ct")
                nc.sync.dma_start(
                    ct[:].rearrange("p (t o) -> p t o", t=nt),
                    codes_r[:, it_lo:it_hi, :],
                )
                st = stp.tile([128, fd], F16, tag="st")
                nc.sync.dma_start(
                    st[:].rearrange("p (t o) -> p t o", t=nt),
                    scales_r[:, it_lo:it_hi, :],
                )
                acc = accp.tile([128, fd], F16, tag="acc")
                nc.vector.tensor_scalar(
                    acc[:], ct[:], DEC_G0, DEC_B, op0=ALU.mult, op1=ALU.add
                )
                # steps all on DVE: is_ge+mult fused pass runs in 2x mode;
                # scalar_tensor_tensor would be ~2x slower at any width
                for v, d in DEC_STEPS:
                    sg = sgp.tile([128, fd], F16, tag="sg")
                    nc.vector.tensor_scalar(
                        sg[:], ct[:], v - 0.5, d,
                        op0=ALU.is_ge, op1=ALU.mult,
                    )
                    nc.vector.tensor_tensor(
                        acc[:], acc[:], sg[:], op=ALU.add
                    )
                for (u, g), bt in zip(DEC_RAMPS, ramp_biases):
                    rp = sgp.tile([128, fd], F16, tag="sg")
                    nc.scalar.activation(
                        rp[:], ct[:], ACTF.Relu, bias=bt[:], scale=abs(g)
                    )
                    nc.vector.tensor_tensor(
                        acc[:], acc[:], rp[:],
                        op=ALU.add if g > 0 else ALU.subtract,
                    )
                # w = acc * scale + W_lora
                wtmp = sgp.tile([128, fd], F16, tag="sg")
                nc.vector.tensor_tensor(wtmp[:], acc[:], st[:], op=ALU.mult)
                wt = wpool.tile([128, fd], BF16, tag=f"w{it_lo}")
                nc.vector.tensor_tensor(wt[:], wtmp[:], wl[:], op=ALU.add)
                for j, it in enumerate(range(it_lo, it_hi)):
                    w_aps[it] = wt[:, j * O_SH:(j + 1) * O_SH]

            for rng in MACROS[:N_IMMEDIATE]:
                emit_macro(*rng)

            # ---- m-loop phases; partial lives in SBUF as bf16 ----
            part_tiles = [
                partp.tile([128, O_SH], BF16, tag=f"pt{mt}", name=f"pt{mt}")
                for mt in range(MT)
            ]
            n_ph = len(PHASES)
            for ph, (lo, hi) in enumerate(PHASES):
                w = hi - lo
                for mt in range(MT):
                    xa = xpool.tile([128, w, 128], BF16, tag="xa")
                    nc.sync.dma_start(xa[:], xt.ap()[:, mt, lo:hi, :])
                    po = pp.tile([128, O_SH], F32, tag="po")
                    for k, it in enumerate(range(lo, hi)):
                        nc.tensor.matmul(
                            po[:], xa[:, k, :], w_aps[it],
                            start=(k == 0), stop=(k == w - 1),
                        )
                    pt = part_tiles[mt]
                    if ph == 0:
                        # ACT handles all phase-0 evacuation (DVE carries
                        # the decode chain during this window)
                        nc.scalar.copy(pt[:], po[:])
                    elif ph < n_ph - 1:
                        nc.vector.tensor_tensor(
                            pt[:], po[:], pt[:], op=ALU.add
                        )
                    else:
                        ev = opool.tile([128, O_SH], F32, tag="ev")
                        nc.vector.tensor_tensor(
                            ev[:], po[:], pt[:], op=ALU.add
                        )
                        nc.sync.dma_start(
                            out.ap()[mt * 128:(mt + 1) * 128, :], ev[:]
                        )
                for rng in MACROS_AFTER_PHASE.get(ph, []):
                    emit_macro(*rng)

    nc.compile()
    return nc


_NC_CACHE = {}


def _get_nc():
    if "nc" not in _NC_CACHE:
        _NC_CACHE["nc"] = _build_nc()
    return _NC_CACHE["nc"]


def prepare_in_maps(x, w_codes, w_scales, lora_a, lora_b):
    """Host-side sharding + layout prep (no arithmetic beyond casts/folds)."""
    xm = np.ascontiguousarray(x.reshape(M, I))
    # xt[p, mt, t, mm] = x[mt*128+mm, t*128+p], bf16
    xtl = (
        xm.T.reshape(IT, 128, MT, 128)
        .transpose(1, 2, 0, 3)
        .astype(BF16_NP)
    )
    xtl = np.ascontiguousarray(xtl)

    la = np.ascontiguousarray(
        (LORA_SCALE * lora_a.astype(np.float64)).astype(BF16_NP)
    )

    in_maps = []
    for c in range(N_CORES):
        o_lo, o_hi = c * O_SH, (c + 1) * O_SH
        codes_t = np.ascontiguousarray(
            w_codes[o_lo:o_hi].T.astype(np.float16)
        )
        scales_t = np.ascontiguousarray(
            np.repeat(w_scales[o_lo:o_hi].T, BLK, axis=0).astype(np.float16)
        )
        lb_t = np.ascontiguousarray(lora_b[o_lo:o_hi].T.astype(BF16_NP))
        in_maps.append(
            {
                "xt": xtl,
                "codes": codes_t,
                "scales": scales_t,
                "la": la,
                "lb": lb_t,
            }
        )
    return in_maps


def run(in_maps, trace=False, retries=2):
    nc = _get_nc()
    last = None
    for attempt in range(retries + 1):
        try:
            return run_bass_kernel_spmd(
                nc, in_maps, core_ids=list(range(N_CORES)), trace=trace
            )
        except Exception as e:  # transient NRT/axon device errors
            last = e
            if attempt == retries:
                raise
            import time as _time

            _time.sleep(5)
    raise last


def kernel(x, w_codes, w_scales, lora_a, lora_b):
    in_maps = prepare_in_maps(x, w_codes, w_scales, lora_a, lora_b)
    res = run(in_maps, trace=False)
    out = np.concatenate(
        [res.results[c]["out"] for c in range(N_CORES)], axis=1
    )
    return out.reshape(B, S, O).astype(np.float32)


# revision 19
# speedup vs baseline: 1.1411x; 1.0052x over previous
"""NF4-quantized LoRA linear layer on 8 Trainium2 NeuronCores.

Computation (reference):
    w = NF4_TABLE[w_codes] * w_scales[block-expanded]        # [O, I]
    out = x @ w.T + (alpha/rank) * (x @ lora_a.T) @ lora_b.T # [B, S, O]

Strategy:
  - Tensor-parallel split of the output dim across 8 cores (O_SH = 512 each).
    Every core sees all of x; no collectives; host concatenates outputs.
  - The LoRA path is folded into the weights once per core:
    W_eff = dequant(codes) * scales + (alpha/rank) * lora_a.T @ lora_b.T,
    so the steady-state loop is a single dense bf16 matmul.
  - NF4 dequant uses an 8-basis approximation (linear + 4 steps + 3 ramps,
    f16-chain-tuned, max table err 4.9e-3): ACT computes Sign/Relu basis
    tiles, DVE merges them with fused scalar_tensor_tensor ops. This keeps
    both engines ~50% loaded instead of serializing everything on DVE.
  - The contraction dim runs in 4 phases (3/8/11/10 i-tiles). The running
    partial stays in SBUF as bf16 (no DRAM round trip); phase psums merge
    into it with one DVE add per m-tile, and the last phase adds + emits
    f32 straight to the output DMA.
"""

import numpy as np
import ml_dtypes

import concourse.mybir as mybir
import concourse.tile as tile
from concourse import bacc
from concourse.bass_utils import run_bass_kernel_spmd

B, S, I, O, R, BLK = 4, 2048, 4096, 4096, 16, 64
M = B * S                      # 8192 token rows
N_CORES = 8
O_SH = O // N_CORES            # 512 output cols per core
IT = I // 128                  # 32 contraction tiles
MT = M // 128                  # 64 row tiles
LORA_SCALE = 2.0               # alpha / rank

# NF4 decode: t(c) ~= G0*c + BP' + sum d_v*[c>=v] + sum g_u*relu(c-u)
# (8 basis incl. linear; constants tuned on the exact f16 chain,
#  max abs table err 4.9e-3 -> ~7e-3 worst-case rel err on the output,
#  comfortably under the 2e-2 gate)
DEC_B = -0.9998779296875
DEC_G0 = 0.3038071990013129
DEC_STEPS = [(2, 0.06993426640908616), (3, 0.03336246990628824),
             (14, 0.04890079195464007), (15, 0.1629870898660809)]
DEC_RAMPS = [(1, -0.20262171586434732), (6, -0.017353153673390995),
             (11, 0.03004063897796238)]
# ramp-heavy variant for the first macros: ACT carries 6 of the 8 basis
# functions, shortening the DVE serial chain that gates phase-0 start
E_B = -0.9990234375
E_G0 = 0.30380719900131414
E_STEPS = [(3, 0.032996258968788515)]
E_RAMPS = [(1, -0.13268744945526192), (2, -0.06999530156533577),
           (6, -0.017353153673391), (11, 0.03004063897796425),
           (13, 0.04890079195463761), (14, 0.11408629791143897)]

PHASES = [(0, 6), (6, 19), (19, 32)]
# dequant macro-tile i-ranges; narrow at the front for low first-weight
# latency, wide later for DVE efficiency
MACROS = [(0, 2), (2, 4), (4, 6),
          (6, 10), (10, 14), (14, 17), (17, 19),
          (19, 24), (24, 28), (28, 32)]
# macros emitted after each phase's loop (the rest are emitted up front);
# deadlines: a phase's tiles must decode before that phase STARTS, so
# tiles 6-18 decode during phase 0 and 19-31 during phase 1
MACROS_AFTER_PHASE = {0: [(6, 10), (10, 14), (14, 17), (17, 19)],
                      1: [(19, 24), (24, 28), (28, 32)]}
N_IMMEDIATE = 3                # macros emitted before phase 0
N_RAMP_HEAVY = 3               # early macros using the E_* chain

F16 = mybir.dt.float16
BF16 = mybir.dt.bfloat16
F32 = mybir.dt.float32
ALU = mybir.AluOpType
ACTF = mybir.ActivationFunctionType

BF16_NP = ml_dtypes.bfloat16


def _build_nc():
    nc = bacc.Bacc("TRN2", target_bir_lowering=False, debug=False,
                   num_devices=N_CORES)

    xt = nc.dram_tensor("xt", [128, MT, IT, 128], BF16, kind="ExternalInput")
    codes = nc.dram_tensor("codes", [I, O_SH], F16, kind="ExternalInput")
    scales = nc.dram_tensor("scales", [I, O_SH], F16, kind="ExternalInput")
    la = nc.dram_tensor("la", [R, I], BF16, kind="ExternalInput")
    lb = nc.dram_tensor("lb", [R, O_SH], BF16, kind="ExternalInput")
    out = nc.dram_tensor("out", [M, O_SH], F32, kind="ExternalOutput")

    codes_r = codes.ap().rearrange("(t p) o -> p t o", p=128)
    scales_r = scales.ap().rearrange("(t p) o -> p t o", p=128)

    BP = DEC_B + sum(d for _, d in DEC_STEPS) / 2.0

    with tile.TileContext(nc) as tc:
        with (
            tc.tile_pool(name="wpool", bufs=1) as wpool,
            tc.tile_pool(name="wlpool", bufs=1) as wlpool,
            tc.tile_pool(name="ctp", bufs=2) as ctp,
            tc.tile_pool(name="stp", bufs=2) as stp,
            tc.tile_pool(name="sgp", bufs=4) as sgp,
            tc.tile_pool(name="accp", bufs=2) as accp,
            tc.tile_pool(name="xpool", bufs=4) as xpool,
            tc.tile_pool(name="partp", bufs=1) as partp,
            tc.tile_pool(name="cpool", bufs=1) as cpool,
            tc.tile_pool(name="opool", bufs=3) as opool,
            tc.tile_pool(name="ps", bufs=6, space="PSUM") as pp,
            tc.tile_pool(name="ps_l", bufs=2, space="PSUM") as pp_l,
        ):
            # constants
            la_sb = cpool.tile([R, I], BF16, tag="la")
            nc.sync.dma_start(la_sb[:], la.ap())
            lb_sb = cpool.tile([R, O_SH], BF16, tag="lb")
            nc.sync.dma_start(lb_sb[:], lb.ap())
            ramp_biases = {}
            for u, g in DEC_RAMPS + E_RAMPS:
                key = (u, round(abs(g) * u, 9))
                if key in ramp_biases:
                    continue
                bt = cpool.tile(
                    [128, 1], F32, tag=f"rb{len(ramp_biases)}",
                    name=f"rb{len(ramp_biases)}",
                )
                nc.vector.memset(bt[:], -abs(g) * u)
                ramp_biases[key] = bt

            w_aps = {}

            # ---- LoRA fold: fold matmuls run before phase 0 (they warm
            # the PE); copies alternate DVE/ACT ----
            wl_tiles = {}

            def emit_fold(macros):
                for it_lo, it_hi in macros:
                    wl = wlpool.tile(
                        [128, (it_hi - it_lo) * O_SH], BF16,
                        tag=f"wl{it_lo}", name=f"wl{it_lo}",
                    )
                    wl_tiles[it_lo] = wl
                    for j, it in enumerate(range(it_lo, it_hi)):
                        pl = pp_l.tile([128, O_SH], F32, tag="pl", name="pl")
                        nc.tensor.matmul(
                            pl[:], la_sb[:, it * 128:(it + 1) * 128],
                            lb_sb[:], start=True, stop=True,
                        )
                        dst = wl[:, j * O_SH:(j + 1) * O_SH]
                        if it % 2 == 0:
                            nc.vector.tensor_scalar_mul(dst, pl[:], 1.0)
                        else:
                            nc.scalar.copy(dst, pl[:])

            def emit_macro(it_lo, it_hi, ramp_heavy=False):
                nt = it_hi - it_lo
                fd = nt * O_SH
                wl = wl_tiles[it_lo]
                b0, g0 = (E_B, E_G0) if ramp_heavy else (DEC_B, DEC_G0)
                steps = E_STEPS if ramp_heavy else DEC_STEPS
                ramps = E_RAMPS if ramp_heavy else DEC_RAMPS
                # NF4 decode
                ct = ctp.tile([128, fd], F16, tag="ct")
                nc.sync.dma_start(
                    ct[:].rearrange("p (t o) -> p t o", t=nt),
                    codes_r[:, it_lo:it_hi, :],
                )
                st = stp.tile([128, fd], F16, tag="st")
                nc.sync.dma_start(
                    st[:].rearrange("p (t o) -> p t o", t=nt),
                    scales_r[:, it_lo:it_hi, :],
                )
                acc = accp.tile([128, fd], F16, tag="acc")
                nc.vector.tensor_scalar(
                    acc[:], ct[:], g0, b0, op0=ALU.mult, op1=ALU.add
                )
                # steps on DVE: is_ge+mult fused pass runs in 2x mode;
                # scalar_tensor_tensor would be ~2x slower at any width
                for v, d in steps:
                    sg = sgp.tile([128, fd], F16, tag="sg")
                    nc.vector.tensor_scalar(
                        sg[:], ct[:], v - 0.5, d,
                        op0=ALU.is_ge, op1=ALU.mult,
                    )
                    nc.vector.tensor_tensor(
                        acc[:], acc[:], sg[:], op=ALU.add
                    )
                for u, g in ramps:
                    bt = ramp_biases[(u, round(abs(g) * u, 9))]
                    rp = sgp.tile([128, fd], F16, tag="sg")
                    nc.scalar.activation(
                        rp[:], ct[:], ACTF.Relu, bias=bt[:], scale=abs(g)
                    )
                    nc.vector.tensor_tensor(
                        acc[:], acc[:], rp[:],
                        op=ALU.add if g > 0 else ALU.subtract,
                    )
                # w = acc * scale + W_lora
                wtmp = sgp.tile([128, fd], F16, tag="sg")
                nc.vector.tensor_tensor(wtmp[:], acc[:], st[:], op=ALU.mult)
                wt = wpool.tile([128, fd], BF16, tag=f"w{it_lo}")
                nc.vector.tensor_tensor(wt[:], wtmp[:], wl[:], op=ALU.add)
                for j, it in enumerate(range(it_lo, it_hi)):
                    w_aps[it] = wt[:, j * O_SH:(j + 1) * O_SH]

            emit_fold(MACROS[:N_IMMEDIATE])
            for mi, rng in enumerate(MACROS[:N_IMMEDIATE]):
                emit_macro(*rng, ramp_heavy=(mi < N_RAMP_HEAVY))
            emit_fold(MACROS[N_IMMEDIATE:])

            # ---- m-loop phases; partial lives in SBUF as bf16 ----
            part_tiles = [
                partp.tile([128, O_SH], BF16, tag=f"pt{mt}", name=f"pt{mt}")
                for mt in range(MT)
            ]
            n_ph = len(PHASES)
            for ph, (lo, hi) in enumerate(PHASES):
                w = hi - lo
                for mt in range(MT):
                    xa = xpool.tile([128, w, 128], BF16, tag="xa")
                    nc.sync.dma_start(xa[:], xt.ap()[:, mt, lo:hi, :])
                    po = pp.tile([128, O_SH], F32, tag="po")
                    for k, it in enumerate(range(lo, hi)):
                        nc.tensor.matmul(
                            po[:], xa[:, k, :], w_aps[it],
                            start=(k == 0), stop=(k == w - 1),
                        )
                    pt = part_tiles[mt]
                    if ph == 0:
                        # ACT handles all phase-0 evacuation (DVE carries
                        # the decode chain during this window)
                        nc.scalar.copy(pt[:], po[:])
                    elif ph < n_ph - 1:
                        nc.vector.tensor_tensor(
                            pt[:], po[:], pt[:], op=ALU.add
                        )
                    else:
                        ev = opool.tile([128, O_SH], F32, tag="ev")
                        nc.vector.tensor_tensor(
                            ev[:], po[:], pt[:], op=ALU.add
                        )
                        nc.sync.dma_start(
                            out.ap()[mt * 128:(mt + 1) * 128, :], ev[:]
                        )
                for rng in MACROS_AFTER_PHASE.get(ph, []):
                    emit_macro(*rng)

    nc.compile()
    return nc


_NC_CACHE = {}


def _get_nc():
    if "nc" not in _NC_CACHE:
        _NC_CACHE["nc"] = _build_nc()
    return _NC_CACHE["nc"]


def prepare_in_maps(x, w_codes, w_scales, lora_a, lora_b):
    """Host-side sharding + layout prep (no arithmetic beyond casts/folds)."""
    xm = np.ascontiguousarray(x.reshape(M, I))
    # xt[p, mt, t, mm] = x[mt*128+mm, t*128+p], bf16
    xtl = (
        xm.T.reshape(IT, 128, MT, 128)
        .transpose(1, 2, 0, 3)
        .astype(BF16_NP)
    )
    xtl = np.ascontiguousarray(xtl)

    la = np.ascontiguousarray(
        (LORA_SCALE * lora_a.astype(np.float64)).astype(BF16_NP)
    )

    in_maps = []
    for c in range(N_CORES):
        o_lo, o_hi = c * O_SH, (c + 1) * O_SH
        codes_t = np.ascontiguousarray(
            w_codes[o_lo:o_hi].T.astype(np.float16)
        )
        scales_t = np.ascontiguousarray(
            np.repeat(w_scales[o_lo:o_hi].T, BLK, axis=0).astype(np.float16)
        )
        lb_t = np.ascontiguousarray(lora_b[o_lo:o_hi].T.astype(BF16_NP))
        in_maps.append(
            {
                "xt": xtl,
                "codes": codes_t,
                "scales": scales_t,
                "la": la,
                "lb": lb_t,
            }
        )
    return in_maps


def run(in_maps, trace=False, retries=2):
    nc = _get_nc()
    last = None
    for attempt in range(retries + 1):
        try:
            return run_bass_kernel_spmd(
                nc, in_maps, core_ids=list(range(N_CORES)), trace=trace
            )
        except Exception as e:  # transient NRT/axon device errors
            last = e
            if attempt == retries:
                raise
            import time as _time

            _time.sleep(5)
    raise last


def kernel(x, w_codes, w_scales, lora_a, lora_b):
    in_maps = prepare_in_maps(x, w_codes, w_scales, lora_a, lora_b)
    res = run(in_maps, trace=False)
    out = np.concatenate(
        [res.results[c]["out"] for c in range(N_CORES)], axis=1
    )
    return out.reshape(B, S, O).astype(np.float32)
